# revision 5
# baseline (speedup 1.0000x reference)
"""Causal multi-head attention on 8 Trainium2 NeuronCores.

Problem: x[4,2048,1024] @ w_qkv[1024,3072] -> 16-head causal attention
         -> ctx @ w_out[1024,1024] + b_out.

Sharding (tensor-parallel heads x data-parallel batch):
  8 cores = 2 batch-groups (2 batches each) x 4 head-groups (4 heads each).
  Each core computes qkv for its 4 heads over its 2 batches, runs causal
  flash-style attention, and emits a partial output projection
  (w_out rows for its heads). Host sums the 4 head-group partials per
  batch-group and adds b_out.

Per-core layout/algorithm (T=2048, HD=64):
  - x is pre-transposed on host: xT[b, p, c8, t] = x[b, t, c8*128+p].
  - QT/KT computed head-PAIR-interleaved: [128 (=2 heads x 64 dims), T],
    via matmul(lhsT=wq_chunk[128c,128], rhs=xT_chunk[128c,512]).
  - V computed in token-major layout [128 tokens, 4*64] and evacuated into
    per-k-block [Ve|1|Vo|1] tiles (the appended ones column makes each PV
    matmul also accumulate the softmax denominators).
  - Scores are computed transposed, S^T[k,q], K=64 matmuls at partition
    bases 0/64 (row-tiled pairs run concurrently in the PE array); 4 heads
    share one [128,1024] PSUM region so a single Exp covers all 4.
  - Causal masking: diagonal k-blocks multiply exp by a 0/1 mask (GPSIMD).
  - PV: out^T accumulation, lhsT=[V|1] (M=65), denominators land in
    partition 64; normalize via reciprocal + DMA-broadcast + DVE muls.
  - Output projection: ctx^T pair tiles as lhsT against w_out row-chunks.

All matmul operands are float32r (TF32-like, 1 cycle/row at N>=256,
measured rel err ~1.5e-4 per matmul chain).
"""
import sys

sys.path.insert(0, "/opt/trn_rl_repo")

import numpy as np
import concourse.bacc as bacc
import concourse.mybir as mybir
import concourse.tile as tile
from concourse.bass_utils import run_bass_kernel_spmd

F32 = mybir.dt.float32
F32R = mybir.dt.float32r

B, T, C = 4, 2048, 1024
NH, HD = 16, 64
BL = 2          # batches per core
HL = 4          # heads per core
NTS = T // 512  # 4 t-slices
NTB = T // 128  # 16 token blocks (= k blocks)
NQT = T // 256  # 8 q tiles
SC = 1.0 / np.sqrt(HD)

# PT4/S4 column offset of each (pair, parity): pair0-even, pair1-even,
# pair0-odd, pair1-odd -> row-tiled pairs (even@rows0:64, odd@rows64:128)
# write different PSUM banks.
COL = {(0, 0): 0, (1, 0): 256, (0, 1): 512, (1, 1): 768}


def build_kernel():
    nc = bacc.Bacc("TRN2", target_bir_lowering=False, debug=False)
    xT = nc.dram_tensor("xT", [BL, 128, 8, T], F32R, kind="ExternalInput").ap()
    wq = nc.dram_tensor("wq", [128, 8, 256], F32R, kind="ExternalInput").ap()
    wk = nc.dram_tensor("wk", [128, 8, 256], F32R, kind="ExternalInput").ap()
    wv = nc.dram_tensor("wv", [128, 8, 256], F32R, kind="ExternalInput").ap()
    wo = nc.dram_tensor("wo", [128, 2, 1024], F32R, kind="ExternalInput").ap()
    mk = nc.dram_tensor("mk", [128, 2048], F32R, kind="ExternalInput").ap()
    out = nc.dram_tensor("out", [BL, T, C], F32, kind="ExternalOutput").ap()

    with tile.TileContext(nc) as tc:
        with (
            tc.tile_pool(name="const", bufs=1) as cpool,
            tc.tile_pool(name="xt", bufs=2) as xpool,
            tc.tile_pool(name="qk", bufs=1) as qkpool,
            tc.tile_pool(name="v", bufs=1) as vpool,
            tc.tile_pool(name="pt", bufs=3) as ptpool,
            tc.tile_pool(name="ctx", bufs=1) as ctxpool,
            tc.tile_pool(name="nrm", bufs=2) as nrmpool,
            tc.tile_pool(name="osb", bufs=3) as opool,
            tc.tile_pool(name="ps_s", bufs=2, space="PSUM") as ps_s,
            tc.tile_pool(name="ps_o", bufs=2, space="PSUM") as ps_o,
            tc.tile_pool(name="ps_w", bufs=2, space="PSUM") as ps_w,
        ):
            wq_sb = cpool.tile([128, 8, 256], F32R, tag="wq")
            wk_sb = cpool.tile([128, 8, 256], F32R, tag="wk")
            wv_sb = cpool.tile([128, 8, 256], F32R, tag="wv")
            wo_sb = cpool.tile([128, 2, 1024], F32R, tag="wo")
            mk_sb = cpool.tile([128, 2048], F32R, tag="mk")
            ones_sb = cpool.tile([128, 1], F32, tag="ones")
            nc.sync.dma_start(wq_sb[:], wq)
            nc.sync.dma_start(wk_sb[:], wk)
            nc.sync.dma_start(wv_sb[:], wv)
            nc.sync.dma_start(wo_sb[:], wo)
            nc.sync.dma_start(mk_sb[:], mk)
            nc.gpsimd.memset(ones_sb[:], 1.0)

            for b in range(BL):
                # ---------------- QKV projection ----------------
                qt_t = {}  # (pair, ts) -> [128, 512] pair-interleaved QT
                kt_t = {}
                v_t = {}   # j -> [128, 260] = [V_e|1|V_o|1] per pair
                for ts in range(NTS):
                    xt = xpool.tile([128, 8, 512], F32R, tag="xt")
                    nc.sync.dma_start(xt[:], xT[b, :, :, ts * 512:(ts + 1) * 512])
                    for pair in range(2):
                        for which, wsb, store in (("q", wq_sb, qt_t), ("k", wk_sb, kt_t)):
                            ps = ps_w.tile([128, 512], F32, tag="w")
                            for c8 in range(8):
                                nc.tensor.matmul(
                                    ps[:],
                                    wsb[:, c8, pair * 128:(pair + 1) * 128],
                                    xt[:, c8, :],
                                    start=(c8 == 0),
                                    stop=(c8 == 7),
                                )
                            t_ = qkpool.tile(
                                [128, 512], F32R, tag=f"{which}{pair}_{ts}"
                            )
                            nc.vector.tensor_copy(t_[:], ps[:])
                            store[(pair, ts)] = t_
                    for tb in range(4):
                        j = ts * 4 + tb
                        ps = ps_w.tile([128, 256], F32, tag="w")
                        for c8 in range(8):
                            nc.tensor.matmul(
                                ps[:],
                                xt[:, c8, tb * 128:(tb + 1) * 128],
                                wv_sb[:, c8, :],
                                start=(c8 == 0),
                                stop=(c8 == 7),
                            )
                        vt = vpool.tile([128, 260], F32R, tag=f"v_{j}")
                        nc.vector.tensor_copy(
                            vt[:].rearrange("p (g c) -> p g c", c=65)[:, :, 0:64],
                            ps[:].rearrange("p (g c) -> p g c", c=64),
                        )
                        nc.vector.tensor_copy(
                            vt[:].rearrange("p (g c) -> p g c", c=65)[:, :, 64:65],
                            ones_sb[:, None, :].to_broadcast((128, 4, 1)),
                        )
                        v_t[j] = vt

                # ---------------- causal attention ----------------
                ctx_t = {}  # (pair, qtile) -> [128, 256] pair-interleaved ctx^T
                for i in range(NQT):
                    # one PSUM bank (= one accumulation group) per pair; the
                    # group spans the whole k-loop, parities share it via
                    # disjoint column ranges.
                    o_ps = {
                        p: ps_o.tile([65, 512], F32, tag="o", name=f"o_ps{p}")
                        for p in range(2)
                    }
                    njb = 2 * i + 2
                    for j in range(njb):
                        s4 = ps_s.tile([128, 1024], F32, tag="s4")
                        for pair in range(2):
                            for par in range(2):  # even rows 0:64 / odd 64:128
                                hs = slice(par * 64, par * 64 + 64)
                                col = COL[(pair, par)]
                                # each [128,512] bank hosts one transient
                                # group: pair0 starts it, pair1 closes it
                                nc.tensor.matmul(
                                    s4[:, col:col + 256],
                                    kt_t[(pair, j // 4)][hs, (j % 4) * 128:(j % 4) * 128 + 128],
                                    qt_t[(pair, i // 2)][hs, (i % 2) * 256:(i % 2) * 256 + 256],
                                    start=(pair == 0),
                                    stop=(pair == 1),
                                )
                        pt4 = ptpool.tile([128, 1024], F32R, tag="pt4")
                        nc.scalar.activation(
                            pt4[:], s4[:], mybir.ActivationFunctionType.Exp
                        )
                        if j >= njb - 2:  # diagonal k-block: causal 0/1 mask
                            moff = 0 if j == njb - 2 else 1024
                            nc.gpsimd.tensor_mul(
                                pt4[:], pt4[:], mk_sb[:, moff:moff + 1024]
                            )
                        for pair in range(2):
                            for par in range(2):
                                vcol = pair * 130 + par * 65
                                nc.tensor.matmul(
                                    o_ps[pair][0:65, par * 256:par * 256 + 256],
                                    v_t[j][:, vcol:vcol + 65],
                                    pt4[:, COL[(pair, par)]:COL[(pair, par)] + 256],
                                    start=(j == 0 and par == 0),
                                    stop=(j == njb - 1 and par == 1),
                                )
                    # normalize: denominators sit in partition 64
                    for pair in range(2):
                        rec = nrmpool.tile([65, 512], F32, tag="rec")
                        nc.vector.reciprocal(rec[64:65, :], o_ps[pair][64:65, :])
                        # partition_broadcast reads physical partition 0, so
                        # stage the reciprocal row there first.
                        nc.sync.dma_start(rec[0:1, :], rec[64:65, :])
                        bc = nrmpool.tile([128, 512], F32, tag="bc")
                        nc.gpsimd.partition_broadcast(bc[:], rec[0:1, :])
                        ct = ctxpool.tile([128, 256], F32R, tag=f"ctx{pair}_{i}")
                        nc.vector.tensor_mul(
                            ct[0:64, :], o_ps[pair][0:64, 0:256], bc[0:64, 0:256]
                        )
                        todd = nrmpool.tile([64, 256], F32R, tag="todd")
                        nc.vector.tensor_mul(
                            todd[0:64, :], o_ps[pair][0:64, 256:512], bc[0:64, 256:512]
                        )
                        nc.sync.dma_start(ct[64:128, :], todd[0:64, :])
                        ctx_t[(pair, i)] = ct

                # ---------------- output projection ----------------
                for tblk in range(NTB):
                    i, half = tblk // 2, (tblk % 2) * 128
                    for nt in range(2):
                        ps = ps_w.tile([128, 512], F32, tag="w")
                        for pair in range(2):
                            nc.tensor.matmul(
                                ps[:],
                                ctx_t[(pair, i)][:, half:half + 128],
                                wo_sb[:, pair, nt * 512:nt * 512 + 512],
                                start=(pair == 0),
                                stop=(pair == 1),
                            )
                        osb = opool.tile([128, 512], F32, tag="osb")
                        nc.vector.tensor_copy(osb[:], ps[:])
                        nc.sync.dma_start(
                            out[b, tblk * 128:(tblk + 1) * 128, nt * 512:(nt + 1) * 512],
                            osb[:],
                        )
    nc.compile()
    return nc


def make_masks():
    kk = np.arange(128)[:, None]
    qq = np.arange(256)[None, :]
    mA = (kk <= qq).astype(np.float32)        # diagonal block j = 2i
    mB = (kk + 128 <= qq).astype(np.float32)  # diagonal block j = 2i+1
    return np.concatenate([np.tile(mA, (1, 4)), np.tile(mB, (1, 4))], axis=1)


def make_in_maps(x, w_qkv, w_out):
    masks = make_masks()
    in_maps = []
    for core in range(8):
        bg, hg = core // 4, core % 4
        xb = x[2 * bg:2 * bg + 2]                       # [2, T, C]
        xt = xb.transpose(0, 2, 1).reshape(BL, 8, 128, T).transpose(0, 2, 1, 3)
        wq_ = (w_qkv[:, hg * 256:(hg + 1) * 256] * SC).reshape(8, 128, 256)
        wk_ = w_qkv[:, C + hg * 256:C + (hg + 1) * 256].reshape(8, 128, 256)
        wv_ = w_qkv[:, 2 * C + hg * 256:2 * C + (hg + 1) * 256].reshape(8, 128, 256)
        wo_ = w_out[hg * 256:(hg + 1) * 256, :].reshape(2, 128, 1024)
        in_maps.append({
            "xT": np.ascontiguousarray(xt, dtype=np.float32),
            "wq": np.ascontiguousarray(wq_.transpose(1, 0, 2), dtype=np.float32),
            "wk": np.ascontiguousarray(wk_.transpose(1, 0, 2), dtype=np.float32),
            "wv": np.ascontiguousarray(wv_.transpose(1, 0, 2), dtype=np.float32),
            "wo": np.ascontiguousarray(wo_.transpose(1, 0, 2), dtype=np.float32),
            "mk": masks,
        })
    return in_maps


_CACHE = {}


def kernel(x, w_qkv, w_out, b_out):
    x = np.asarray(x, dtype=np.float32)
    w_qkv = np.asarray(w_qkv, dtype=np.float32)
    w_out = np.asarray(w_out, dtype=np.float32)
    b_out = np.asarray(b_out, dtype=np.float32)

    if "nc" not in _CACHE:
        _CACHE["nc"] = build_kernel()
    nc = _CACHE["nc"]

    in_maps = make_in_maps(x, w_qkv, w_out)
    res = run_bass_kernel_spmd(nc, in_maps, core_ids=list(range(8)))

    out = np.empty((B, T, C), dtype=np.float32)
    for bg in range(2):
        acc = res.results[4 * bg]["out"].astype(np.float32).copy()
        for hg in range(1, 4):
            acc += res.results[4 * bg + hg]["out"]
        out[2 * bg:2 * bg + 2] = acc + b_out[None, None, :]
    return out


# revision 7
# speedup vs baseline: 240.0102x; 240.0102x over previous
"""Causal multi-head attention on 8 Trainium2 NeuronCores.

Problem: x[4,2048,1024] @ w_qkv[1024,3072] -> 16-head causal attention
         -> ctx @ w_out[1024,1024] + b_out.

Sharding (tensor-parallel heads x data-parallel batch):
  8 cores = 2 batch-groups (2 batches each) x 4 head-groups (4 heads each).
  Each core computes qkv for its 4 heads over its 2 batches, runs causal
  flash-style attention, and emits a partial output projection
  (w_out rows for its heads). Host sums the 4 head-group partials per
  batch-group and adds b_out.

Per-core layout/algorithm (T=2048, HD=64):
  - x is pre-transposed on host: xT[b, p, c8, t] = x[b, t, c8*128+p].
  - QT/KT computed head-PAIR-interleaved: [128 (=2 heads x 64 dims), T],
    via matmul(lhsT=wq_chunk[128c,128], rhs=xT_chunk[128c,512]).
  - V computed in token-major layout [128 tokens, 4*64] and evacuated into
    per-k-block [Ve|1|Vo|1] tiles (the appended ones column makes each PV
    matmul also accumulate the softmax denominators).
  - Scores are computed transposed, S^T[k,q], K=64 matmuls at partition
    bases 0/64 (row-tiled pairs run concurrently in the PE array); 4 heads
    share one [128,1024] PSUM region so a single Exp covers all 4.
  - Causal masking: diagonal k-blocks multiply exp by a 0/1 mask (GPSIMD).
  - PV: out^T accumulation, lhsT=[V|1] (M=65), denominators land in
    partition 64; normalize via reciprocal + DMA-broadcast + DVE muls.
  - Output projection: ctx^T pair tiles as lhsT against w_out row-chunks.

All matmul operands are float32r (TF32-like, 1 cycle/row at N>=256,
measured rel err ~1.5e-4 per matmul chain).
"""
import sys

sys.path.insert(0, "/opt/trn_rl_repo")

import numpy as np
import concourse.bacc as bacc
import concourse.mybir as mybir
import concourse.tile as tile
from concourse.bass_utils import run_bass_kernel_spmd

F32 = mybir.dt.float32
F32R = mybir.dt.float32r

B, T, C = 4, 2048, 1024
NH, HD = 16, 64
BL = 2          # batches per core
HL = 4          # heads per core
NTS = T // 512  # 4 t-slices
NTB = T // 128  # 16 token blocks (= k blocks)
NQT = T // 256  # 8 q tiles
SC = 1.0 / np.sqrt(HD)

# PT4/S4 column offset of each (pair, parity): pair0-even, pair1-even,
# pair0-odd, pair1-odd -> row-tiled pairs (even@rows0:64, odd@rows64:128)
# write different PSUM banks.
COL = {(0, 0): 0, (1, 0): 256, (0, 1): 512, (1, 1): 768}


def build_kernel(reps=1):
    nc = bacc.Bacc("TRN2", target_bir_lowering=False, debug=False)
    xT = nc.dram_tensor("xT", [BL, 128, 8, T], F32R, kind="ExternalInput").ap()
    wq = nc.dram_tensor("wq", [128, 8, 256], F32R, kind="ExternalInput").ap()
    wk = nc.dram_tensor("wk", [128, 8, 256], F32R, kind="ExternalInput").ap()
    wv = nc.dram_tensor("wv", [128, 8, 256], F32R, kind="ExternalInput").ap()
    wo = nc.dram_tensor("wo", [128, 2, 1024], F32R, kind="ExternalInput").ap()
    mk = nc.dram_tensor("mk", [128, 2048], F32R, kind="ExternalInput").ap()
    out = nc.dram_tensor("out", [BL, T, C], F32, kind="ExternalOutput").ap()

    with tile.TileContext(nc) as tc:
        with (
            tc.tile_pool(name="const", bufs=1) as cpool,
            tc.tile_pool(name="xt", bufs=2) as xpool,
            tc.tile_pool(name="qk", bufs=1) as qkpool,
            tc.tile_pool(name="v", bufs=1) as vpool,
            tc.tile_pool(name="pt", bufs=3) as ptpool,
            tc.tile_pool(name="ctx", bufs=1) as ctxpool,
            tc.tile_pool(name="nrm", bufs=2) as nrmpool,
            tc.tile_pool(name="osb", bufs=3) as opool,
            tc.tile_pool(name="ps_s", bufs=2, space="PSUM") as ps_s,
            tc.tile_pool(name="ps_o", bufs=2, space="PSUM") as ps_o,
            tc.tile_pool(name="ps_w", bufs=2, space="PSUM") as ps_w,
        ):
            wq_sb = cpool.tile([128, 8, 256], F32R, tag="wq")
            wk_sb = cpool.tile([128, 8, 256], F32R, tag="wk")
            wv_sb = cpool.tile([128, 8, 256], F32R, tag="wv")
            wo_sb = cpool.tile([128, 2, 1024], F32R, tag="wo")
            mk_sb = cpool.tile([128, 2048], F32R, tag="mk")
            ones_sb = cpool.tile([128, 1], F32, tag="ones")
            nc.sync.dma_start(wq_sb[:], wq)
            nc.sync.dma_start(wk_sb[:], wk)
            nc.sync.dma_start(wv_sb[:], wv)
            nc.sync.dma_start(wo_sb[:], wo)
            nc.sync.dma_start(mk_sb[:], mk)
            nc.gpsimd.memset(ones_sb[:], 1.0)

            for b in [b for _ in range(reps) for b in range(BL)]:
                # ---------------- QKV projection ----------------
                qt_t = {}  # (pair, ts) -> [128, 512] pair-interleaved QT
                kt_t = {}
                v_t = {}   # j -> [128, 260] = [V_e|1|V_o|1] per pair
                for ts in range(NTS):
                    xt = xpool.tile([128, 8, 512], F32R, tag="xt")
                    nc.sync.dma_start(xt[:], xT[b, :, :, ts * 512:(ts + 1) * 512])
                    for pair in range(2):
                        for which, wsb, store in (("q", wq_sb, qt_t), ("k", wk_sb, kt_t)):
                            ps = ps_w.tile([128, 512], F32, tag="w")
                            for c8 in range(8):
                                nc.tensor.matmul(
                                    ps[:],
                                    wsb[:, c8, pair * 128:(pair + 1) * 128],
                                    xt[:, c8, :],
                                    start=(c8 == 0),
                                    stop=(c8 == 7),
                                )
                            t_ = qkpool.tile(
                                [128, 512], F32R, tag=f"{which}{pair}_{ts}"
                            )
                            nc.vector.tensor_copy(t_[:], ps[:])
                            store[(pair, ts)] = t_
                    for tb in range(4):
                        j = ts * 4 + tb
                        ps = ps_w.tile([128, 256], F32, tag="w")
                        for c8 in range(8):
                            nc.tensor.matmul(
                                ps[:],
                                xt[:, c8, tb * 128:(tb + 1) * 128],
                                wv_sb[:, c8, :],
                                start=(c8 == 0),
                                stop=(c8 == 7),
                            )
                        vt = vpool.tile([128, 260], F32R, tag=f"v_{j}")
                        nc.vector.tensor_copy(
                            vt[:].rearrange("p (g c) -> p g c", c=65)[:, :, 0:64],
                            ps[:].rearrange("p (g c) -> p g c", c=64),
                        )
                        nc.vector.tensor_copy(
                            vt[:].rearrange("p (g c) -> p g c", c=65)[:, :, 64:65],
                            ones_sb[:, None, :].to_broadcast((128, 4, 1)),
                        )
                        v_t[j] = vt

                # ---------------- causal attention ----------------
                ctx_t = {}  # (pair, qtile) -> [128, 256] pair-interleaved ctx^T
                for i in range(NQT):
                    # one PSUM bank (= one accumulation group) per pair; the
                    # group spans the whole k-loop, parities share it via
                    # disjoint column ranges.
                    o_ps = {
                        p: ps_o.tile([65, 512], F32, tag="o", name=f"o_ps{p}")
                        for p in range(2)
                    }
                    njb = 2 * i + 2
                    for j in range(njb):
                        s4 = ps_s.tile([128, 1024], F32, tag="s4")
                        for pair in range(2):
                            for par in range(2):  # even rows 0:64 / odd 64:128
                                hs = slice(par * 64, par * 64 + 64)
                                col = COL[(pair, par)]
                                # each [128,512] bank hosts one transient
                                # group: pair0 starts it, pair1 closes it
                                nc.tensor.matmul(
                                    s4[:, col:col + 256],
                                    kt_t[(pair, j // 4)][hs, (j % 4) * 128:(j % 4) * 128 + 128],
                                    qt_t[(pair, i // 2)][hs, (i % 2) * 256:(i % 2) * 256 + 256],
                                    start=(pair == 0),
                                    stop=(pair == 1),
                                )
                        pt4 = ptpool.tile([128, 1024], F32R, tag="pt4")
                        nc.scalar.activation(
                            pt4[:], s4[:], mybir.ActivationFunctionType.Exp
                        )
                        if j >= njb - 2:  # diagonal k-block: causal 0/1 mask
                            moff = 0 if j == njb - 2 else 1024
                            nc.gpsimd.tensor_mul(
                                pt4[:], pt4[:], mk_sb[:, moff:moff + 1024]
                            )
                        for pair in range(2):
                            for par in range(2):
                                vcol = pair * 130 + par * 65
                                nc.tensor.matmul(
                                    o_ps[pair][0:65, par * 256:par * 256 + 256],
                                    v_t[j][:, vcol:vcol + 65],
                                    pt4[:, COL[(pair, par)]:COL[(pair, par)] + 256],
                                    start=(j == 0 and par == 0),
                                    stop=(j == njb - 1 and par == 1),
                                )
                    # normalize: denominators sit in partition 64
                    for pair in range(2):
                        rec = nrmpool.tile([65, 512], F32, tag="rec")
                        nc.vector.reciprocal(rec[64:65, :], o_ps[pair][64:65, :])
                        # partition_broadcast reads physical partition 0, so
                        # stage the reciprocal row there first.
                        nc.sync.dma_start(rec[0:1, :], rec[64:65, :])
                        bc = nrmpool.tile([128, 512], F32, tag="bc")
                        nc.gpsimd.partition_broadcast(bc[:], rec[0:1, :])
                        ct = ctxpool.tile([128, 256], F32R, tag=f"ctx{pair}_{i}")
                        nc.vector.tensor_mul(
                            ct[0:64, :], o_ps[pair][0:64, 0:256], bc[0:64, 0:256]
                        )
                        todd = nrmpool.tile([64, 256], F32R, tag="todd")
                        nc.vector.tensor_mul(
                            todd[0:64, :], o_ps[pair][0:64, 256:512], bc[0:64, 256:512]
                        )
                        nc.sync.dma_start(ct[64:128, :], todd[0:64, :])
                        ctx_t[(pair, i)] = ct

                # ---------------- output projection ----------------
                for tblk in range(NTB):
                    i, half = tblk // 2, (tblk % 2) * 128
                    for nt in range(2):
                        ps = ps_w.tile([128, 512], F32, tag="w")
                        for pair in range(2):
                            nc.tensor.matmul(
                                ps[:],
                                ctx_t[(pair, i)][:, half:half + 128],
                                wo_sb[:, pair, nt * 512:nt * 512 + 512],
                                start=(pair == 0),
                                stop=(pair == 1),
                            )
                        osb = opool.tile([128, 512], F32, tag="osb")
                        nc.vector.tensor_copy(osb[:], ps[:])
                        nc.sync.dma_start(
                            out[b, tblk * 128:(tblk + 1) * 128, nt * 512:(nt + 1) * 512],
                            osb[:],
                        )
    nc.compile()
    return nc


def make_masks():
    kk = np.arange(128)[:, None]
    qq = np.arange(256)[None, :]
    mA = (kk <= qq).astype(np.float32)        # diagonal block j = 2i
    mB = (kk + 128 <= qq).astype(np.float32)  # diagonal block j = 2i+1
    return np.concatenate([np.tile(mA, (1, 4)), np.tile(mB, (1, 4))], axis=1)


def make_in_maps(x, w_qkv, w_out):
    masks = make_masks()
    in_maps = []
    for core in range(8):
        bg, hg = core // 4, core % 4
        xb = x[2 * bg:2 * bg + 2]                       # [2, T, C]
        xt = xb.transpose(0, 2, 1).reshape(BL, 8, 128, T).transpose(0, 2, 1, 3)
        wq_ = (w_qkv[:, hg * 256:(hg + 1) * 256] * SC).reshape(8, 128, 256)
        wk_ = w_qkv[:, C + hg * 256:C + (hg + 1) * 256].reshape(8, 128, 256)
        wv_ = w_qkv[:, 2 * C + hg * 256:2 * C + (hg + 1) * 256].reshape(8, 128, 256)
        wo_ = w_out[hg * 256:(hg + 1) * 256, :].reshape(2, 128, 1024)
        in_maps.append({
            "xT": np.ascontiguousarray(xt, dtype=np.float32),
            "wq": np.ascontiguousarray(wq_.transpose(1, 0, 2), dtype=np.float32),
            "wk": np.ascontiguousarray(wk_.transpose(1, 0, 2), dtype=np.float32),
            "wv": np.ascontiguousarray(wv_.transpose(1, 0, 2), dtype=np.float32),
            "wo": np.ascontiguousarray(wo_.transpose(1, 0, 2), dtype=np.float32),
            "mk": masks,
        })
    return in_maps


_CACHE = {}


def kernel(x, w_qkv, w_out, b_out):
    x = np.asarray(x, dtype=np.float32)
    w_qkv = np.asarray(w_qkv, dtype=np.float32)
    w_out = np.asarray(w_out, dtype=np.float32)
    b_out = np.asarray(b_out, dtype=np.float32)

    if "nc" not in _CACHE:
        _CACHE["nc"] = build_kernel()
    nc = _CACHE["nc"]

    in_maps = make_in_maps(x, w_qkv, w_out)
    res = run_bass_kernel_spmd(nc, in_maps, core_ids=list(range(8)))

    out = np.empty((B, T, C), dtype=np.float32)
    for bg in range(2):
        acc = res.results[4 * bg]["out"].astype(np.float32).copy()
        for hg in range(1, 4):
            acc += res.results[4 * bg + hg]["out"]
        out[2 * bg:2 * bg + 2] = acc + b_out[None, None, :]
    return out
